# revision 45
# baseline (speedup 1.0000x reference)
# Gaussian-kernel ridge-regression matvec on 8 Trainium2 cores.
#
#   out_i = sum_j exp(-||x_i - y_j||^2 / g) * alpha_j
#   N=8192 queries, M=16384 train points, DIM=32, g scalar.
#
# Factorization (host prep is O(N+M), device does the O(N*M) part):
#   exp(-(x^2+y^2-2xy)/g)*a_j = exp(-x_i^2/g) * sign(a_j) * exp(s_ij),
#   s_ij = (2/g) x_i.y_j + c_j,   c_j = -y_j^2/g + ln|a_j|
# Train points are host-sorted so sign(a)>0 comes first (npos); the device
# computes s via an augmented K=34 fp16 matmul: rows 0-31 are the (2/g)-scaled
# y dims (x rows are the fp16 query dims), rows 32/33 carry c_j split hi/lo
# in fp16 (x rows 32/33 are 1.0) so c reaches the exp at full accuracy while
# the dot itself is single-pass fp16 (~4e-3 abs err in s, well inside the
# 2e-2 gate). The exp runs on ACT in-place on PSUM with accum_out giving
# per-row sums per pos/neg segment; tiny DVE reduce/sub; one DMA out.
# Row scale exp(-x_i^2/g) is applied on host.
#
# ACT is the bottleneck (1 elem/cycle/lane @1.2GHz): 16K j x 1K i per core
# = 131072 elems/lane ~ 109us + ~281-cycle/instr pipeline fill. Structure
# keeps ACT saturated: uniform [128,2048] PSUM groups double-buffered (4+4
# banks), flat (group, itile) iteration so psum-slot reuse never waits a
# bunched itile tail, y streamed in 1024-col chunks over the three
# DMA-capable queues (sync/gpsimd/scalar) so compute starts ~8us in. The
# pos/neg boundary group keeps a uniform full-width ACT (mixed-size ACT
# streams get scheduler-misordered); its neg suffix is folded into the pos
# column and corrected via a DVE reduce of the exp'd suffix from bf16
# scratch: res = possum - negsum - 2*S.

import numpy as np

N, M, DIM, NCORES = 8192, 16384, 32, 8
NLOC = N // NCORES
ITILES = NLOC // 128
GRP = 2048
NGRP = M // GRP
KAUG = DIM + 2
MMN = 512
YCH = 1024

_cache = {}


def _build(npos):
    import concourse.bass as bass
    import concourse.tile as tile
    from concourse import bacc, mybir

    f32 = mybir.dt.float32
    f16 = mybir.dt.float16
    Exp = mybir.ActivationFunctionType.Exp
    X = mybir.AxisListType.X

    nc = bacc.Bacc("TRN2", target_bir_lowering=False, debug=False)
    yt = nc.dram_tensor("yt", [KAUG, M], f16, kind="ExternalInput").ap()
    xt = nc.dram_tensor("xt", [KAUG, NLOC], f16, kind="ExternalInput").ap()
    o = nc.dram_tensor("o", [128, ITILES], f32, kind="ExternalOutput").ap()

    # One uniform [128, GRP] ACT instr per group (the scheduler mis-orders
    # mixed-size ACT streams, stalling matmuls on deferred slot releases).
    # The group containing the pos/neg boundary accumulates its neg suffix
    # into the "pos" column; a DVE reduce of the exp'd suffix straight from
    # PSUM corrects it:  res = possum - negsum - 2*S.
    bg = npos // GRP          # boundary group (== NGRP means all-pos)
    suf0 = npos % GRP         # suffix start within group bg (0 -> clean split)
    npos_grps = bg + (1 if suf0 else 0)
    nneg_grps = NGRP - npos_grps

    with tile.TileContext(nc) as tc:
        with tc.tile_pool(name="ypool", bufs=1) as ypool, \
             tc.tile_pool(name="xpool", bufs=1) as xpool, \
             tc.tile_pool(name="psum", bufs=2, space="PSUM") as pp, \
             tc.tile_pool(name="parts", bufs=2 * ITILES) as partp, \
             tc.tile_pool(name="small", bufs=3 * ITILES + 2) as smallp, \
             tc.tile_pool(name="bscr", bufs=3) as bscrp, \
             tc.tile_pool(name="res", bufs=1) as resp:

            # x first (needed by every matmul), then y chunks spread over the
            # three DMA-capable queues (sync, gpsimd, scalar) so the first
            # matmul group's data lands as early as possible while the rest
            # stream in behind it. Matmuls carry the DMA waits directly.
            # One whole transfer per startup tensor, each FIRST on its own
            # queue: the first matmul then carries exactly two unsatisfied
            # waits (chunk0 + x). Splitting these across queues backfires —
            # the scheduler coalesces >2 waits into one later semaphore
            # value and the matmul over-waits on unrelated chunks. DMA
            # trigger instrs themselves cost ~600-750ns serially per queue,
            # so late chunks ride the scalar queue to shorten the chains.
            # x first (needed by every matmul), then y chunks spread over the
            # three DMA-capable queues (sync, gpsimd, scalar) so the first
            # matmul group's data lands as early as possible while the rest
            # stream in behind it. Matmuls carry the DMA waits directly.
            xts = xpool.tile([KAUG, NLOC], f16, tag="xt")
            nc.scalar.dma_start(xts[:], xt[:])
            queues = [nc.sync, nc.gpsimd]
            yts = []
            for ci in range(M // YCH):
                t = ypool.tile([KAUG, YCH], f16, tag=f"yt{ci}", name=f"ytile{ci}")
                if ci == 0:
                    # chunk 0 in thirds across all three queues
                    nc.sync.dma_start(t[:, 0:384], yt[:, 0:384])
                    nc.gpsimd.dma_start(t[:, 384:768], yt[:, 384:768])
                    nc.scalar.dma_start(t[:, 768:YCH], yt[:, 768:YCH])
                elif ci == 1:
                    for hh in range(2):
                        queues[hh].dma_start(
                            t[:, bass.ts(hh, YCH // 2)],
                            yt[:, bass.ts(ci * 2 + hh, YCH // 2)])
                else:
                    queues[ci % 2].dma_start(t[:], yt[:, bass.ts(ci, YCH)])
                yts.append(t)

            # Warm the ACT exp table during the DMA wait (the first real
            # ACTIVATE otherwise eats the ~1.3us ACT_TABLE_LOAD). Emitted
            # after the scalar-queue DMA triggers so those fire first.
            dact = smallp.tile([1, 1], f32, tag="dact")
            nc.vector.memset(dact[:], 0.0)
            nc.scalar.activation(dact[:], dact[:], Exp)
            # fp32 SBUF scratch for groups whose reduction runs on DVE
            # instead of the ACT accumulator (drops the READ_ACCUMULATOR
            # tail from the ACT queue) and for the boundary group (keeps
            # the DVE suffix-reduce off the PSUM slot's release path).
            bscr = [bscrp.tile([128, GRP], f32, tag=f"bscr{i}", name=f"bscr{i}")
                    for i in range(3)]

            res = resp.tile([128, ITILES], f32)
            # separate pos/neg partial tiles per itile: the pos reduction
            # (emitted right after the last pos group) must not create a
            # tile-granularity read hazard against later neg READ_ACCUMULATOR
            # writes, which would stall the ACT queue mid-stream.
            partsp = [partp.tile([128, max(npos_grps, 1)], f32,
                                 tag=f"pp{it}", name=f"pp{it}")
                      for it in range(ITILES)]
            partsn = [partp.tile([128, max(nneg_grps, 1)], f32,
                                 tag=f"pn{it}", name=f"pn{it}")
                      for it in range(ITILES)]
            possums = [smallp.tile([128, 1], f32, tag=f"pos{it}", name=f"pos{it}")
                       for it in range(ITILES)]
            negsums = [smallp.tile([128, 1], f32, tag=f"neg{it}", name=f"neg{it}")
                       for it in range(ITILES)]
            sufs = [smallp.tile([128, 1], f32, tag=f"suf{it}", name=f"suf{it}")
                    for it in range(ITILES)]

            for gi in range(NGRP):
                g0 = gi * GRP
                for it in range(ITILES):
                    xw = xts[:, bass.ts(it, 128)]
                    ps = pp.tile([128, GRP], f32, tag="ps")
                    for h in range(GRP // MMN):
                        j0 = g0 + h * MMN
                        ci, off = j0 // YCH, j0 % YCH
                        nc.tensor.matmul(ps[:, bass.ts(h, MMN)], xw,
                                         yts[ci][:, off:off + MMN],
                                         start=True, stop=True)
                    if gi < npos_grps:
                        tgt = partsp[it][:, gi:gi + 1]
                    else:
                        tgt = partsn[it][:, gi - npos_grps:gi - npos_grps + 1]
                    if gi == bg and suf0:
                        # boundary group: exp to SBUF scratch so the DVE
                        # suffix-reduce doesn't hold the PSUM slot
                        sc = bscr[it % 2]
                        nc.scalar.activation(sc[:], ps[:], Exp, accum_out=tgt)
                        nc.vector.reduce_sum(sufs[it][:], sc[:, suf0:GRP],
                                             axis=X)
                    else:
                        nc.scalar.activation(ps[:], ps[:], Exp, accum_out=tgt)
                    # partial reductions as soon as an itile's pos (or neg)
                    # columns are complete — keeps the tail to one itile.
                    if gi == npos_grps - 1 and npos_grps:
                        nc.vector.reduce_sum(possums[it][:], partsp[it][:],
                                             axis=X)
                    if gi == NGRP - 1:
                        if nneg_grps:
                            nc.vector.reduce_sum(negsums[it][:], partsn[it][:],
                                                 axis=X)
                        else:
                            nc.vector.memset(negsums[it][:], 0.0)
                        if not npos_grps:
                            nc.vector.memset(possums[it][:], 0.0)
                        nc.vector.tensor_sub(res[:, it:it + 1],
                                             possums[it][:], negsums[it][:])
                        if suf0:
                            nc.vector.tensor_sub(res[:, it:it + 1],
                                                 res[:, it:it + 1],
                                                 sufs[it][:])
                            nc.vector.tensor_sub(res[:, it:it + 1],
                                                 res[:, it:it + 1],
                                                 sufs[it][:])

            nc.sync.dma_start(o[:], res[:])

    nc.compile()
    return nc


def kernel(x, y_train, alphas, g):
    from concourse.bass_utils import run_bass_kernel_spmd

    x = np.asarray(x, dtype=np.float32)
    y_train = np.asarray(y_train, dtype=np.float32)
    a = np.asarray(alphas, dtype=np.float32).reshape(-1)
    gf = float(np.asarray(g).reshape(-1)[0])

    y2 = np.sum(y_train.astype(np.float64) ** 2, axis=1)
    with np.errstate(divide="ignore"):
        c = -y2 / gf + np.log(np.abs(a.astype(np.float64)))
    c = np.maximum(c, -1e4)

    pos = a >= 0
    order = np.concatenate([np.nonzero(pos)[0], np.nonzero(~pos)[0]])
    npos = int(pos.sum())

    ytab = np.empty((KAUG, M), dtype=np.float64)
    ytab[:DIM] = (2.0 / gf) * y_train[order].T.astype(np.float64)
    co = c[order]
    ch = co.astype(np.float16).astype(np.float64)
    ytab[DIM] = ch
    ytab[DIM + 1] = co - ch
    ytn = ytab.astype(np.float16)

    key = npos
    if key not in _cache:
        _cache[key] = _build(npos)
    nc = _cache[key]

    in_maps = []
    for k in range(NCORES):
        xs = x[k * NLOC:(k + 1) * NLOC]
        xtab = np.empty((KAUG, NLOC), dtype=np.float64)
        xtab[:DIM] = xs.T.astype(np.float64)
        xtab[DIM] = 1.0
        xtab[DIM + 1] = 1.0
        in_maps.append({
            "yt": ytn,
            "xt": xtab.astype(np.float16),
        })

    r = run_bass_kernel_spmd(nc, in_maps, core_ids=list(range(NCORES)))

    x2 = np.sum(x.astype(np.float64) ** 2, axis=1)
    rowscale = np.exp(-x2 / gf)
    out = np.empty(N, dtype=np.float64)
    for k in range(NCORES):
        out[k * NLOC:(k + 1) * NLOC] = r.results[k]["o"].T.reshape(NLOC).astype(np.float64)
    out *= rowscale
    return out.astype(np.float32).reshape(N, 1)


# revision 49
# speedup vs baseline: 1.0097x; 1.0097x over previous
# Gaussian-kernel ridge-regression matvec on 8 Trainium2 cores.
#
#   out_i = sum_j exp(-||x_i - y_j||^2 / g) * alpha_j
#   N=8192 queries, M=16384 train points, DIM=32, g scalar.
#
# Factorization (host prep is O(N+M), device does the O(N*M) part):
#   exp(-(x^2+y^2-2xy)/g)*a_j = exp(-x_i^2/g) * sign(a_j) * exp(s_ij),
#   s_ij = (2/g) x_i.y_j + c_j,   c_j = -y_j^2/g + ln|a_j|
# Train points are host-sorted so sign(a)>0 comes first (npos); the device
# computes s via an augmented K=34 fp16 matmul: rows 0-31 are the (2/g)-scaled
# y dims (x rows are the fp16 query dims), rows 32/33 carry c_j split hi/lo
# in fp16 (x rows 32/33 are 1.0) so c reaches the exp at full accuracy while
# the dot itself is single-pass fp16 (~4e-3 abs err in s, well inside the
# 2e-2 gate). The exp runs on ACT in-place on PSUM with accum_out giving
# per-row sums per pos/neg segment; tiny DVE reduce/sub; one DMA out.
# Row scale exp(-x_i^2/g) is applied on host.
#
# ACT is the bottleneck (1 elem/cycle/lane @1.2GHz): 16K j x 1K i per core
# = 131072 elems/lane ~ 109us + ~281-cycle/instr pipeline fill. Structure
# keeps ACT saturated: uniform [128,2048] PSUM groups double-buffered (4+4
# banks), flat (group, itile) iteration so psum-slot reuse never waits a
# bunched itile tail, y streamed in 1024-col chunks over the three
# DMA-capable queues (sync/gpsimd/scalar) so compute starts ~8us in. The
# pos/neg boundary group keeps a uniform full-width ACT (mixed-size ACT
# streams get scheduler-misordered); its neg suffix is folded into the pos
# column and corrected via a DVE reduce of the exp'd suffix from bf16
# scratch: res = possum - negsum - 2*S.

import numpy as np

N, M, DIM, NCORES = 8192, 16384, 32, 8
NLOC = N // NCORES
ITILES = NLOC // 128
GRP = 2048
NGRP = M // GRP
KAUG = DIM + 2
MMN = 512
YCH = 1024

_cache = {}


def _build(npos):
    import concourse.bass as bass
    import concourse.tile as tile
    from concourse import bacc, mybir

    f32 = mybir.dt.float32
    f16 = mybir.dt.float16
    Exp = mybir.ActivationFunctionType.Exp
    X = mybir.AxisListType.X

    nc = bacc.Bacc("TRN2", target_bir_lowering=False, debug=False)
    yt = nc.dram_tensor("yt", [KAUG, M], f16, kind="ExternalInput").ap()
    xt = nc.dram_tensor("xt", [KAUG, NLOC], f16, kind="ExternalInput").ap()
    o = nc.dram_tensor("o", [128, ITILES], f32, kind="ExternalOutput").ap()

    # One uniform [128, GRP] ACT instr per group (the scheduler mis-orders
    # mixed-size ACT streams, stalling matmuls on deferred slot releases).
    # The group containing the pos/neg boundary accumulates its neg suffix
    # into the "pos" column; a DVE reduce of the exp'd suffix straight from
    # PSUM corrects it:  res = possum - negsum - 2*S.
    bg = npos // GRP          # boundary group (== NGRP means all-pos)
    suf0 = npos % GRP         # suffix start within group bg (0 -> clean split)
    npos_grps = bg + (1 if suf0 else 0)
    nneg_grps = NGRP - npos_grps

    with tile.TileContext(nc) as tc:
        with tc.tile_pool(name="ypool", bufs=1) as ypool, \
             tc.tile_pool(name="xpool", bufs=1) as xpool, \
             tc.tile_pool(name="psum", bufs=2, space="PSUM") as pp, \
             tc.tile_pool(name="parts", bufs=2 * ITILES) as partp, \
             tc.tile_pool(name="small", bufs=3 * ITILES + 2) as smallp, \
             tc.tile_pool(name="bscr", bufs=3) as bscrp, \
             tc.tile_pool(name="res", bufs=1) as resp:

            # x first (needed by every matmul), then y chunks spread over the
            # three DMA-capable queues (sync, gpsimd, scalar) so the first
            # matmul group's data lands as early as possible while the rest
            # stream in behind it. Matmuls carry the DMA waits directly.
            # One whole transfer per startup tensor, each FIRST on its own
            # queue: the first matmul then carries exactly two unsatisfied
            # waits (chunk0 + x). Splitting these across queues backfires —
            # the scheduler coalesces >2 waits into one later semaphore
            # value and the matmul over-waits on unrelated chunks. DMA
            # trigger instrs themselves cost ~600-750ns serially per queue,
            # so late chunks ride the scalar queue to shorten the chains.
            # x on the scalar queue; the first matmul group's 2048 cols as
            # four 512-col tiles (one DMA dependency per matmul — no wait
            # coalescing, ~68KB critical bytes per queue); later chunks as
            # 1024-col tiles alternating sync/gpsimd behind them.
            xts = xpool.tile([KAUG, NLOC], f16, tag="xt")
            nc.scalar.dma_start(xts[:], xt[:])
            y0s = []
            for h in range(4):
                t = ypool.tile([KAUG, MMN], f16, tag=f"y0{h}", name=f"y0t{h}")
                y0s.append(t)
            queues = [nc.sync, nc.gpsimd]
            for h in (0, 2, 1, 3):
                queues[h % 2].dma_start(y0s[h][:], yt[:, bass.ts(h, MMN)])
            yts = [None, None]
            for ci in range(2, M // YCH):
                t = ypool.tile([KAUG, YCH], f16, tag=f"yt{ci}", name=f"ytile{ci}")
                queues[ci % 2].dma_start(t[:], yt[:, bass.ts(ci, YCH)])
                yts.append(t)

            # Explicit zero-bias AP for every activation: the default float
            # bias is lowered to a const-pool AP whose TENSOR_LOAD sits in
            # front of the DMA triggers on the queues at startup.
            zb = smallp.tile([128, 1], f32, tag="zb")
            nc.vector.memset(zb[:], 0.0)
            # Warm the ACT exp table during the DMA wait (the first real
            # ACTIVATE otherwise eats the ~1.3us ACT_TABLE_LOAD). Emitted
            # after the scalar-queue DMA triggers so those fire first.
            dact = smallp.tile([1, 1], f32, tag="dact")
            nc.vector.memset(dact[:], 0.0)
            nc.scalar.activation(dact[:], dact[:], Exp, bias=zb[0:1, :])
            # fp32 SBUF scratch for groups whose reduction runs on DVE
            # instead of the ACT accumulator (drops the READ_ACCUMULATOR
            # tail from the ACT queue) and for the boundary group (keeps
            # the DVE suffix-reduce off the PSUM slot's release path).
            bscr = [bscrp.tile([128, GRP], f32, tag=f"bscr{i}", name=f"bscr{i}")
                    for i in range(3)]

            res = resp.tile([128, ITILES], f32)
            # separate pos/neg partial tiles per itile: the pos reduction
            # (emitted right after the last pos group) must not create a
            # tile-granularity read hazard against later neg READ_ACCUMULATOR
            # writes, which would stall the ACT queue mid-stream.
            partsp = [partp.tile([128, max(npos_grps, 1)], f32,
                                 tag=f"pp{it}", name=f"pp{it}")
                      for it in range(ITILES)]
            partsn = [partp.tile([128, max(nneg_grps, 1)], f32,
                                 tag=f"pn{it}", name=f"pn{it}")
                      for it in range(ITILES)]
            possums = [smallp.tile([128, 1], f32, tag=f"pos{it}", name=f"pos{it}")
                       for it in range(ITILES)]
            negsums = [smallp.tile([128, 1], f32, tag=f"neg{it}", name=f"neg{it}")
                       for it in range(ITILES)]
            sufs = [smallp.tile([128, 1], f32, tag=f"suf{it}", name=f"suf{it}")
                    for it in range(ITILES)]

            for gi in range(NGRP):
                g0 = gi * GRP
                for it in range(ITILES):
                    xw = xts[:, bass.ts(it, 128)]
                    ps = pp.tile([128, GRP], f32, tag="ps")
                    for h in range(GRP // MMN):
                        j0 = g0 + h * MMN
                        if gi == 0:
                            rhs = y0s[h][:]
                        else:
                            ci, off = j0 // YCH, j0 % YCH
                            rhs = yts[ci][:, off:off + MMN]
                        nc.tensor.matmul(ps[:, bass.ts(h, MMN)], xw, rhs,
                                         start=True, stop=True)
                    if gi < npos_grps:
                        tgt = partsp[it][:, gi:gi + 1]
                    else:
                        tgt = partsn[it][:, gi - npos_grps:gi - npos_grps + 1]
                    if gi == bg and suf0:
                        # boundary group: exp to SBUF scratch so the DVE
                        # suffix-reduce doesn't hold the PSUM slot
                        sc = bscr[it % 2]
                        nc.scalar.activation(sc[:], ps[:], Exp, bias=zb[:],
                                             accum_out=tgt)
                        nc.vector.reduce_sum(sufs[it][:], sc[:, suf0:GRP],
                                             axis=X)
                    else:
                        nc.scalar.activation(ps[:], ps[:], Exp, bias=zb[:],
                                             accum_out=tgt)
                    # partial reductions as soon as an itile's pos (or neg)
                    # columns are complete — keeps the tail to one itile.
                    if gi == npos_grps - 1 and npos_grps:
                        nc.vector.reduce_sum(possums[it][:], partsp[it][:],
                                             axis=X)
                    if gi == NGRP - 1:
                        if nneg_grps:
                            nc.vector.reduce_sum(negsums[it][:], partsn[it][:],
                                                 axis=X)
                        else:
                            nc.vector.memset(negsums[it][:], 0.0)
                        if not npos_grps:
                            nc.vector.memset(possums[it][:], 0.0)
                        nc.vector.tensor_sub(res[:, it:it + 1],
                                             possums[it][:], negsums[it][:])
                        if suf0:
                            nc.vector.tensor_sub(res[:, it:it + 1],
                                                 res[:, it:it + 1],
                                                 sufs[it][:])
                            nc.vector.tensor_sub(res[:, it:it + 1],
                                                 res[:, it:it + 1],
                                                 sufs[it][:])

            nc.sync.dma_start(o[:], res[:])

    nc.compile()
    return nc


def kernel(x, y_train, alphas, g):
    from concourse.bass_utils import run_bass_kernel_spmd

    x = np.asarray(x, dtype=np.float32)
    y_train = np.asarray(y_train, dtype=np.float32)
    a = np.asarray(alphas, dtype=np.float32).reshape(-1)
    gf = float(np.asarray(g).reshape(-1)[0])

    y2 = np.sum(y_train.astype(np.float64) ** 2, axis=1)
    with np.errstate(divide="ignore"):
        c = -y2 / gf + np.log(np.abs(a.astype(np.float64)))
    c = np.maximum(c, -1e4)

    pos = a >= 0
    order = np.concatenate([np.nonzero(pos)[0], np.nonzero(~pos)[0]])
    npos = int(pos.sum())

    ytab = np.empty((KAUG, M), dtype=np.float64)
    ytab[:DIM] = (2.0 / gf) * y_train[order].T.astype(np.float64)
    co = c[order]
    ch = co.astype(np.float16).astype(np.float64)
    ytab[DIM] = ch
    ytab[DIM + 1] = co - ch
    ytn = ytab.astype(np.float16)

    key = npos
    if key not in _cache:
        _cache[key] = _build(npos)
    nc = _cache[key]

    in_maps = []
    for k in range(NCORES):
        xs = x[k * NLOC:(k + 1) * NLOC]
        xtab = np.empty((KAUG, NLOC), dtype=np.float64)
        xtab[:DIM] = xs.T.astype(np.float64)
        xtab[DIM] = 1.0
        xtab[DIM + 1] = 1.0
        in_maps.append({
            "yt": ytn,
            "xt": xtab.astype(np.float16),
        })

    r = run_bass_kernel_spmd(nc, in_maps, core_ids=list(range(NCORES)))

    x2 = np.sum(x.astype(np.float64) ** 2, axis=1)
    rowscale = np.exp(-x2 / gf)
    out = np.empty(N, dtype=np.float64)
    for k in range(NCORES):
        out[k * NLOC:(k + 1) * NLOC] = r.results[k]["o"].T.reshape(NLOC).astype(np.float64)
    out *= rowscale
    return out.astype(np.float32).reshape(N, 1)


# revision 52
# speedup vs baseline: 1.0179x; 1.0081x over previous
# Gaussian-kernel ridge-regression matvec on 8 Trainium2 cores.
#
#   out_i = sum_j exp(-||x_i - y_j||^2 / g) * alpha_j
#   N=8192 queries, M=16384 train points, DIM=32, g scalar.
#
# Factorization (host prep is O(N+M), device does the O(N*M) part):
#   exp(-(x^2+y^2-2xy)/g)*a_j = exp(-x_i^2/g) * sign(a_j) * exp(s_ij),
#   s_ij = (2/g) x_i.y_j + c_j,   c_j = -y_j^2/g + ln|a_j|
# Train points are host-sorted so sign(a)>0 comes first (npos); the device
# computes s via an augmented K=34 fp16 matmul: rows 0-31 are the (2/g)-scaled
# y dims (x rows are the fp16 query dims), rows 32/33 carry c_j split hi/lo
# in fp16 (x rows 32/33 are 1.0) so c reaches the exp at full accuracy while
# the dot itself is single-pass fp16 (~4e-3 abs err in s, well inside the
# 2e-2 gate). The exp runs on ACT in-place on PSUM with accum_out giving
# per-row sums per pos/neg segment; tiny DVE reduce/sub; one DMA out.
# Row scale exp(-x_i^2/g) is applied on host.
#
# ACT is the bottleneck (1 elem/cycle/lane @1.2GHz): 16K j x 1K i per core
# = 131072 elems/lane ~ 109us + ~281-cycle/instr pipeline fill. Structure
# keeps ACT saturated: uniform [128,2048] PSUM groups double-buffered (4+4
# banks), flat (group, itile) iteration so psum-slot reuse never waits a
# bunched itile tail, y streamed in chunks over the three DMA-capable
# queues (sync/gpsimd/scalar) so compute starts ~12us in. The
# pos/neg boundary group keeps a uniform full-width ACT (mixed-size ACT
# streams get scheduler-misordered); its neg suffix is folded into the pos
# column and corrected via a DVE reduce of the exp'd suffix from bf16
# scratch: res = possum - negsum - 2*S.

import numpy as np

N, M, DIM, NCORES = 8192, 16384, 32, 8
NLOC = N // NCORES
ITILES = NLOC // 128
GRP = 2048
NGRP = M // GRP
KAUG = DIM + 2
MMN = 512
YCH = 1024

_cache = {}


def _build(npos):
    import concourse.bass as bass
    import concourse.tile as tile
    from concourse import bacc, mybir

    f32 = mybir.dt.float32
    f16 = mybir.dt.float16
    Exp = mybir.ActivationFunctionType.Exp
    X = mybir.AxisListType.X

    nc = bacc.Bacc("TRN2", target_bir_lowering=False, debug=False)
    yt = nc.dram_tensor("yt", [KAUG, M], f16, kind="ExternalInput").ap()
    xt = nc.dram_tensor("xt", [KAUG, NLOC], f16, kind="ExternalInput").ap()
    o = nc.dram_tensor("o", [128, ITILES], f32, kind="ExternalOutput").ap()

    # One uniform [128, GRP] ACT instr per group (the scheduler mis-orders
    # mixed-size ACT streams, stalling matmuls on deferred slot releases).
    # The group containing the pos/neg boundary accumulates its neg suffix
    # into the "pos" column; a DVE reduce of the exp'd suffix straight from
    # PSUM corrects it:  res = possum - negsum - 2*S.
    bg = npos // GRP          # boundary group (== NGRP means all-pos)
    suf0 = npos % GRP         # suffix start within group bg (0 -> clean split)
    npos_grps = bg + (1 if suf0 else 0)
    nneg_grps = NGRP - npos_grps

    with tile.TileContext(nc) as tc:
        with tc.tile_pool(name="ypool", bufs=1) as ypool, \
             tc.tile_pool(name="xpool", bufs=1) as xpool, \
             tc.tile_pool(name="psum", bufs=2, space="PSUM") as pp, \
             tc.tile_pool(name="parts", bufs=2 * ITILES) as partp, \
             tc.tile_pool(name="small", bufs=3 * ITILES + 2) as smallp, \
             tc.tile_pool(name="bscr", bufs=3) as bscrp, \
             tc.tile_pool(name="res", bufs=1) as resp:

            # Startup DMA plan: x on the scalar queue; the first matmul
            # group's 2048 cols as four 512-col tiles so each cold-start
            # matmul depends on exactly ONE DMA semaphore (a tile fed by
            # multiple DMAs makes consumers carry >2 waits, which the
            # scheduler coalesces into one LATER semaphore value — the
            # matmul then over-waits on unrelated chunks). Later chunks are
            # 1024-col tiles alternating sync/gpsimd; their waits are long
            # satisfied by the time they're consumed. DMA trigger instrs
            # cost ~600-750ns serially on the issuing queue.
            xts = xpool.tile([KAUG, NLOC], f16, tag="xt")
            nc.scalar.dma_start(xts[:], xt[:])
            y0s = []
            for h in range(4):
                t = ypool.tile([KAUG, MMN], f16, tag=f"y0{h}", name=f"y0t{h}")
                y0s.append(t)
            queues = [nc.sync, nc.gpsimd]
            for h in (0, 2, 1, 3):
                queues[h % 2].dma_start(y0s[h][:], yt[:, bass.ts(h, MMN)])
            yts = [None, None]
            for ci in range(2, M // YCH):
                t = ypool.tile([KAUG, YCH], f16, tag=f"yt{ci}", name=f"ytile{ci}")
                queues[ci % 2].dma_start(t[:], yt[:, bass.ts(ci, YCH)])
                yts.append(t)

            # Explicit zero-bias AP for every activation: the default float
            # bias is lowered to a const-pool AP whose TENSOR_LOAD sits in
            # front of the DMA triggers on the queues at startup.
            zb = smallp.tile([128, 1], f32, tag="zb")
            nc.vector.memset(zb[:], 0.0)
            # Warm the ACT exp table during the DMA wait (the first real
            # ACTIVATE otherwise eats the ~1.3us ACT_TABLE_LOAD). Emitted
            # after the scalar-queue DMA triggers so those fire first.
            dact = smallp.tile([1, 1], f32, tag="dact")
            nc.vector.memset(dact[:], 0.0)
            nc.scalar.activation(dact[:], dact[:], Exp, bias=zb[0:1, :])
            # fp32 SBUF scratch for the boundary group's exp output: keeps
            # the DVE suffix-reduce off the PSUM slot's release path.
            bscr = [bscrp.tile([128, GRP], f32, tag=f"bscr{i}", name=f"bscr{i}")
                    for i in range(3)]

            res = resp.tile([128, ITILES], f32)
            # separate pos/neg partial tiles per itile: the pos reduction
            # (emitted right after the last pos group) must not create a
            # tile-granularity read hazard against later neg READ_ACCUMULATOR
            # writes, which would stall the ACT queue mid-stream.
            partsp = [partp.tile([128, max(npos_grps, 1)], f32,
                                 tag=f"pp{it}", name=f"pp{it}")
                      for it in range(ITILES)]
            partsn = [partp.tile([128, max(nneg_grps, 1)], f32,
                                 tag=f"pn{it}", name=f"pn{it}")
                      for it in range(ITILES)]
            possums = [smallp.tile([128, 1], f32, tag=f"pos{it}", name=f"pos{it}")
                       for it in range(ITILES)]
            negsums = [smallp.tile([128, 1], f32, tag=f"neg{it}", name=f"neg{it}")
                       for it in range(ITILES)]
            sufs = [smallp.tile([128, 1], f32, tag=f"suf{it}", name=f"suf{it}")
                    for it in range(ITILES)]

            for gi in range(NGRP):
                g0 = gi * GRP
                for it in range(ITILES):
                    xw = xts[:, bass.ts(it, 128)]
                    ps = pp.tile([128, GRP], f32, tag="ps")
                    for h in range(GRP // MMN):
                        j0 = g0 + h * MMN
                        if gi == 0:
                            rhs = y0s[h][:]
                        else:
                            ci, off = j0 // YCH, j0 % YCH
                            rhs = yts[ci][:, off:off + MMN]
                        nc.tensor.matmul(ps[:, bass.ts(h, MMN)], xw, rhs,
                                         start=True, stop=True)
                    if gi < npos_grps:
                        tgt = partsp[it][:, gi:gi + 1]
                    else:
                        tgt = partsn[it][:, gi - npos_grps:gi - npos_grps + 1]
                    if gi == bg and suf0:
                        # boundary group: exp to SBUF scratch so the DVE
                        # suffix-reduce doesn't hold the PSUM slot
                        sc = bscr[it % 2]
                        nc.scalar.activation(sc[:], ps[:], Exp, bias=zb[:],
                                             accum_out=tgt)
                        nc.vector.reduce_sum(sufs[it][:], sc[:, suf0:GRP],
                                             axis=X)
                    else:
                        nc.scalar.activation(ps[:], ps[:], Exp, bias=zb[:],
                                             accum_out=tgt)
                    # partial reductions as soon as an itile's pos (or neg)
                    # columns are complete — keeps the tail to one itile.
                    if gi == npos_grps - 1 and npos_grps:
                        nc.vector.reduce_sum(possums[it][:], partsp[it][:],
                                             axis=X)
                    if gi == NGRP - 1:
                        if nneg_grps:
                            nc.vector.reduce_sum(negsums[it][:], partsn[it][:],
                                                 axis=X)
                        else:
                            nc.vector.memset(negsums[it][:], 0.0)
                        if not npos_grps:
                            nc.vector.memset(possums[it][:], 0.0)
                        nc.vector.tensor_sub(res[:, it:it + 1],
                                             possums[it][:], negsums[it][:])
                        if suf0:
                            nc.vector.tensor_sub(res[:, it:it + 1],
                                                 res[:, it:it + 1],
                                                 sufs[it][:])
                            nc.vector.tensor_sub(res[:, it:it + 1],
                                                 res[:, it:it + 1],
                                                 sufs[it][:])

            nc.sync.dma_start(o[:], res[:])

    nc.compile()
    return nc


def kernel(x, y_train, alphas, g):
    from concourse.bass_utils import run_bass_kernel_spmd

    x = np.asarray(x, dtype=np.float32)
    y_train = np.asarray(y_train, dtype=np.float32)
    a = np.asarray(alphas, dtype=np.float32).reshape(-1)
    gf = float(np.asarray(g).reshape(-1)[0])

    y2 = np.sum(y_train.astype(np.float64) ** 2, axis=1)
    with np.errstate(divide="ignore"):
        c = -y2 / gf + np.log(np.abs(a.astype(np.float64)))
    c = np.maximum(c, -1e4)

    pos = a >= 0
    order = np.concatenate([np.nonzero(pos)[0], np.nonzero(~pos)[0]])
    npos = int(pos.sum())

    ytab = np.empty((KAUG, M), dtype=np.float64)
    ytab[:DIM] = (2.0 / gf) * y_train[order].T.astype(np.float64)
    co = c[order]
    ch = co.astype(np.float16).astype(np.float64)
    ytab[DIM] = ch
    ytab[DIM + 1] = co - ch
    ytn = ytab.astype(np.float16)

    key = npos
    if key not in _cache:
        _cache[key] = _build(npos)
    nc = _cache[key]

    in_maps = []
    for k in range(NCORES):
        xs = x[k * NLOC:(k + 1) * NLOC]
        xtab = np.empty((KAUG, NLOC), dtype=np.float64)
        xtab[:DIM] = xs.T.astype(np.float64)
        xtab[DIM] = 1.0
        xtab[DIM + 1] = 1.0
        in_maps.append({
            "yt": ytn,
            "xt": xtab.astype(np.float16),
        })

    r = run_bass_kernel_spmd(nc, in_maps, core_ids=list(range(NCORES)))

    x2 = np.sum(x.astype(np.float64) ** 2, axis=1)
    rowscale = np.exp(-x2 / gf)
    out = np.empty(N, dtype=np.float64)
    for k in range(NCORES):
        out[k * NLOC:(k + 1) * NLOC] = r.results[k]["o"].T.reshape(NLOC).astype(np.float64)
    out *= rowscale
    return out.astype(np.float32).reshape(N, 1)
